# revision 16
# baseline (speedup 1.0000x reference)
"""MultiHeadCrossAttention Trainium2 Bass kernel (v2).

Sharding (8 cores): data-parallel over batch (2) x tensor-parallel over
head groups (4 groups of 4 heads).  Core c handles batch c//4, heads
4*(c%4) .. 4*(c%4)+3.  Each core computes a partial [Tq, D] output
(its heads' contribution through its Wo row-slice); the host sums the 4
partials per batch.

Device math per core (all matmuls fp16 x fp16 -> fp32 PSUM):
  qT = Wq_s.T @ Xq.T          [256, Tq]   (head-dim on partitions)
  kT = Wk_s.T @ Xkv.T         [256, Tk]
  V  = Xkv @ Wv_s             [Tk, 256]   (+ ones column per head)
  St = kT_h.T @ qT_h          [Tk, Tq] scores^T, K=64, head pairs packed
                              into PE row-groups 0-63 / 64-127
  E  = exp(St/8)              (ScalarE, scale folded into activation)
  P  = E * expb               expb = exp(bias^T) * mask^T  (host-built;
                              multiplicative bias: exp(s+b) = exp(s)exp(b))
  [out^T; sums] = [V_h|1].T @ P   [65, Tq]  ones-column gives softmax sums
  out_norm^T = out^T * (1/sums)   (recip_approx_fast on the PSUM sums row,
                                   gpsimd partition_broadcast, fp16 mul)
  partial = stack(out_norm^T).T @ Wo_s      [Tq, D]  (fp16 out, host sums)

v2 changes vs baseline:
  - host pre-layout: eb shipped [pair, chunk, t, 128, hh*CH] so each
    (pair,chunk,t) is one [128, 2048] DMA with 4KB-contiguous partition
    lines (was 2KB strided); weights likewise partition-major.
  - DMA issue order is first-needed-first; xq split per (dt, tq-half) so
    the first scores group's deps land early.
  - emission: kT -> qT(half 0) -> scores(pair0,chunk0) -> qT(half 1) ->
    V -> attnV(p0,c0) -> scores(p1,c0) -> ... so ACT (exp, the critical
    ~146us engine) starts ~20us earlier.
  - normalize: reciprocal_approx_fast directly on the PSUM sums row (no
    DMA roundtrips / slow reciprocal), fp16 partition_broadcast, stack
    multiply on gpsimd.
  - eb multiply split DVE (hh=0) / gpsimd (hh=1) to keep DVE below ACT.
  - last group runs per-head so attnV/normalize/out-proj overlap the
    final exp stream (shorter tail).

Softmax max-subtraction is skipped: logits ~ N(0, ~1.1), max |logit| < ~7
over 16M samples, exp stays in fp16/fp32 range comfortably.
"""

import os
from contextlib import ExitStack

import numpy as np

import concourse.bass as bass
import concourse.mybir as mybir
import concourse.tile as tile
from concourse import bacc
from concourse.bass_utils import run_bass_kernel_spmd

# Problem dims (hardcoded per contract).
D_MODEL = 1024
NUM_HEADS = 16
D_HEAD = 64
B = 2
TQ = 2048
TK = 2048
N_CORES = 8
HPC = 4  # heads per core
SCALE = 1.0 / 8.0  # 1/sqrt(D_HEAD)

F16 = mybir.dt.float16
F32 = mybir.dt.float32
NP_F16 = np.float16

NQ = 512   # matmul moving free-dim chunk (PSUM bank = 512 fp32)
CH = 1024  # tq chunk per scores tile (2 PSUM banks)


def build_nc(d_model=D_MODEL, tq=TQ, tk=TK, hpc=HPC, d_head=D_HEAD, scale=SCALE):
    """Build the single-core Bass program (SPMD: same NEFF on all cores)."""
    ndt = d_model // 128          # contraction tiles for projections
    pairs = hpc // 2              # head pairs (128 head-dims per pair)
    hd = hpc * d_head             # per-core head dims (= 256)
    ntk = tk // 128               # Tk tiles of 128
    vw = d_head + 1               # V columns per head incl. ones column
    nch = tq // CH                # tq chunks (2)
    groups = [(p, c) for c in range(nch) for p in range(pairs)]

    nc = bacc.Bacc("TRN2", target_bir_lowering=False, debug=False)

    xq_d = nc.dram_tensor("xqT", [d_model, tq], F16, kind="ExternalInput")
    xkv_d = nc.dram_tensor("xkvT", [d_model, tk], F16, kind="ExternalInput")
    wq_d = nc.dram_tensor("wq", [128, ndt, hd], F16, kind="ExternalInput")
    wk_d = nc.dram_tensor("wk", [128, ndt, hd], F16, kind="ExternalInput")
    wv_d = nc.dram_tensor("wv", [128, ndt, hd], F16, kind="ExternalInput")
    wo_d = nc.dram_tensor("wo", [128, pairs, d_model], F16, kind="ExternalInput")
    # [pair, chunk, t, partition(=tk%128), hh*CH]
    eb_d = nc.dram_tensor("expb", [pairs, nch, ntk, 128, 2 * CH], F16,
                          kind="ExternalInput")
    out_d = nc.dram_tensor("out", [tq, d_model], F16, kind="ExternalOutput")

    with ExitStack() as ctx:
        tc = ctx.enter_context(tile.TileContext(nc))
        wpool = ctx.enter_context(tc.tile_pool(name="wpool", bufs=1))
        qkpool = ctx.enter_context(tc.tile_pool(name="qkpool", bufs=1))
        xpool = ctx.enter_context(tc.tile_pool(name="xpool", bufs=1))
        # p tiles [128, CH] f16; also serves the xq half-tiles (same size)
        ppool = ctx.enter_context(tc.tile_pool(name="ppool", bufs=34))
        ebpool = ctx.enter_context(tc.tile_pool(name="ebpool", bufs=8))
        npool = ctx.enter_context(tc.tile_pool(name="npool", bufs=2))
        opool = ctx.enter_context(tc.tile_pool(name="opool", bufs=3))
        psS = ctx.enter_context(tc.tile_pool(name="psS", bufs=3, space="PSUM"))
        psO = ctx.enter_context(tc.tile_pool(name="psO", bufs=2, space="PSUM"))

        wq_sb = wpool.tile([128, ndt, hd], F16, tag="wq")
        wk_sb = wpool.tile([128, ndt, hd], F16, tag="wk")
        wv_sb = wpool.tile([128, ndt, hd], F16, tag="wv")
        wo_sb = wpool.tile([128, pairs, d_model], F16, tag="wo")

        qT_sb = qkpool.tile([128, pairs, tq], F16, tag="qT")
        kT_sb = qkpool.tile([128, pairs, tk], F16, tag="kT")
        v_sb = qkpool.tile([128, ntk, hpc * vw], F16, tag="v")
        stack_sb = qkpool.tile([128, pairs, tq], F16, tag="stack")

        xkv_sb = [xpool.tile([128, tk], F16, tag=f"xkv{dt}", name="xkv_sb")
                  for dt in range(ndt)]
        # xq as [dt][half] tiles of [128, CH] from ppool (same slot size as p)
        xq_sb = [[None] * nch for _ in range(ndt)]

        # ---- DMA issue order: xkv rides the SP queue; weights + xq ride the
        # scalar engine's queue (ACT is idle at startup) so the startup
        # x-stream uses two HWDGE queues in parallel.
        nc.scalar.dma_start(out=wk_sb[:], in_=wk_d.ap())
        nc.scalar.dma_start(out=wq_sb[:], in_=wq_d.ap())
        for dt in range(ndt):
            nc.sync.dma_start(out=xkv_sb[dt][:], in_=xkv_d[dt * 128:(dt + 1) * 128, :])
            xq_sb[dt][0] = ppool.tile([128, CH], F16, tag="p", name="xq_sb")
            nc.scalar.dma_start(out=xq_sb[dt][0][:], in_=xq_d[dt * 128:(dt + 1) * 128, 0:CH])
        nc.sync.dma_start(out=wv_sb[:], in_=wv_d.ap())
        nc.sync.dma_start(out=wo_sb[:], in_=wo_d.ap())

        # ones columns of v_sb (projection copies overwrite the V columns)
        nc.gpsimd.memset(v_sb[:], 1.0)

        # ---- dt-streamed prologue: kT pair0 (both tk halves) accumulates
        # at xkv-DMA pace (2 live chains), then qT pair0 (first tq half).
        ps_k0 = [psS.tile([128, CH], F32, tag="ps", name="ps") for _ in range(2)]
        for dt in range(ndt):
            st, sp = (dt == 0), (dt == ndt - 1)
            for c0 in range(2):
                for q0 in range(0, CH, NQ):
                    nc.tensor.matmul(
                        ps_k0[c0][:, q0:q0 + NQ],
                        wk_sb[:, dt, 0:128],
                        xkv_sb[dt][:, c0 * CH + q0:c0 * CH + q0 + NQ],
                        start=st, stop=sp,
                    )
        for c0 in range(2):
            nc.vector.tensor_copy(kT_sb[:, 0, c0 * CH:(c0 + 1) * CH], ps_k0[c0][:])
        ps_q0 = psS.tile([128, CH], F32, tag="ps", name="ps")
        for dt in range(ndt):
            for q0 in range(0, CH, NQ):
                nc.tensor.matmul(
                    ps_q0[:, q0:q0 + NQ],
                    wq_sb[:, dt, 0:128],
                    xq_sb[dt][0][:, q0:q0 + NQ],
                    start=(dt == 0), stop=(dt == ndt - 1),
                )
        nc.vector.tensor_copy(qT_sb[:, 0, 0:CH], ps_q0[:])

        # ---- generic projection chain (inputs already resident)
        def proj_qk(wsb, xsb_of_dt, dst, j, c0):
            ps = psS.tile([128, CH], F32, tag="ps", name="ps")
            for dt in range(ndt):
                xsb, xoff = xsb_of_dt(dt, c0)
                for q0 in range(0, CH, NQ):
                    nc.tensor.matmul(
                        ps[:, q0:q0 + NQ],
                        wsb[:, dt, j * 128:(j + 1) * 128],
                        xsb[:, xoff + q0:xoff + q0 + NQ],
                        start=(dt == 0),
                        stop=(dt == ndt - 1),
                    )
            nc.vector.tensor_copy(dst[:, j, c0:c0 + CH], ps[:])

        # eb stream is split across the SP queue (even t) and the gpsimd
        # queue (odd t): two descriptor generators, and neither queue's
        # trigger batch sits behind long dependent-compute stalls.
        eb_tiles = {}

        def issue_eb(pair, c):
            for t in range(ntk):
                eb_t = ebpool.tile([128, 2 * CH], F16, tag="eb", name="eb")
                eng = nc.sync if t % 2 == 0 else nc.gpsimd
                eng.dma_start(out=eb_t[:], in_=eb_d[pair, c, t])
                eb_tiles[(pair, c, t)] = eb_t

        # Hold the eb stream until the prologue x-stream lands (a gpsimd op
        # reading the last xq tile gates the queue head): both ride the same
        # DMA engines, and the x-stream is the critical startup path.
        ebgate = npool.tile([1, 8], F16, tag="gate", name="ebgate", bufs=1)
        nc.gpsimd.tensor_copy(ebgate[:], xq_sb[ndt - 1][0][0:1, 0:8])
        issue_eb(0, 0)

        # ---- scores + exp + eb-mul for one (pair, chunk, hh-subset)
        p_tiles = {}

        def scores_group(pair, c, hhs=(0, 1)):
            for t in range(ntk):
                eb_t = eb_tiles[(pair, c, t)]
                for hh in hhs:
                    r0 = hh * 64
                    psA = psS.tile([128, CH], F32, tag="ps", name="ps")
                    for q0 in range(0, CH, NQ):
                        nc.tensor.matmul(
                            psA[:, q0:q0 + NQ],
                            kT_sb[r0:r0 + 64, pair, t * 128:(t + 1) * 128],
                            qT_sb[r0:r0 + 64, pair, c * CH + q0:c * CH + q0 + NQ],
                            start=True,
                            stop=True,
                        )
                    p_t = ppool.tile([128, CH], F16, tag="p", name="p_t")
                    nc.scalar.activation(
                        out=p_t[:], in_=psA[:],
                        func=mybir.ActivationFunctionType.Exp, scale=scale,
                    )
                    nc.vector.tensor_mul(p_t[:], p_t[:], eb_t[:, hh * CH:(hh + 1) * CH])
                    p_tiles[(pair, c, t, hh)] = p_t

        # ---- attnV + normalize for one (pair, chunk, hh)
        def attnv_norm(pair, c, hh):
            h = 2 * pair + hh
            r0 = hh * 64
            for qi in range(CH // NQ):
                po = psO.tile([vw, NQ], F32, tag="po", name="po")
                for t in range(ntk):
                    nc.tensor.matmul(
                        po[:],
                        v_sb[:, t, h * vw:(h + 1) * vw],
                        p_tiles[(pair, c, t, hh)][:, qi * NQ:(qi + 1) * NQ],
                        start=(t == 0),
                        stop=(t == ntk - 1),
                    )
                sm = npool.tile([1, NQ], F32, tag="sm", name="sm")
                nc.vector.tensor_copy(sm[:], po[64:65, :])
                smr = npool.tile([1, NQ], F32, tag="smr", name="smr")
                nc.vector.reciprocal_approx_fast(out=smr[:], in_=sm[:])
                rb = npool.tile([64, NQ], F32, tag="rb", name="rb")
                nc.gpsimd.partition_broadcast(rb[:], smr[:])
                u = npool.tile([64, NQ], F16, tag="u", name="u")
                nc.vector.tensor_copy(u[:], po[0:64, :])
                qg = c * (CH // NQ) + qi
                nc.gpsimd.tensor_mul(
                    stack_sb[r0:r0 + 64, pair, qg * NQ:(qg + 1) * NQ],
                    u[:],
                    rb[:],
                )

        def drop_p(pair, c, hh):
            for t in range(ntk):
                del p_tiles[(pair, c, t, hh)]

        # ---- out-projection for tq tiles [t0, t1)
        def outproj(t0, t1, copy_eng=None):
            for t in range(t0, t1):
                osb = opool.tile([128, d_model], F16, tag="osb", name="osb")
                for mc0 in range(0, d_model, NQ):
                    pf = psO.tile([128, NQ], F32, tag="po", name="pf")
                    for pair in range(pairs):
                        nc.tensor.matmul(
                            pf[:],
                            stack_sb[:, pair, t * 128:(t + 1) * 128],
                            wo_sb[:, pair, mc0:mc0 + NQ],
                            start=(pair == 0),
                            stop=(pair == pairs - 1),
                        )
                    if copy_eng is nc.scalar:
                        nc.scalar.copy(osb[:, mc0:mc0 + NQ], pf[:])
                    else:
                        nc.vector.tensor_copy(osb[:, mc0:mc0 + NQ], pf[:])
                nc.sync.dma_start(out=out_d[t * 128:(t + 1) * 128, :], in_=osb[:])

        # ---- V projection for one head-pair: [tk 128, 128] = X_kv @ Wv_pair
        def vproj(t, pr):
            psv = psO.tile([128, 128], F32, tag="po", name="psv")
            for dt in range(ndt):
                nc.tensor.matmul(
                    psv[:],
                    xkv_sb[dt][:, t * 128:(t + 1) * 128],
                    wv_sb[:, dt, pr * 128:(pr + 1) * 128],
                    start=(dt == 0),
                    stop=(dt == ndt - 1),
                )
            nc.vector.tensor_copy(
                v_sb[:, t, 2 * pr * vw:(2 * pr + 2) * vw]
                    .rearrange("p (h w) -> p h w", w=vw)[:, :, 0:d_head],
                psv[:].rearrange("p (h w) -> p h w", w=d_head),
            )

        # ================= main schedule =================
        # PE order keeps ACT fed with a scores group every ~36us while the
        # filler work (remaining projections, V, out-proj) rides between a
        # group's scores and its attnV (which must wait on exp/mul anyway).
        scores_group(0, 0)
        # second-half xq / remaining weight DMAs (SP queue, after x-stream)
        for dt in range(ndt):
            xq_sb[dt][1] = ppool.tile([128, CH], F16, tag="p", name="xq_sb")
            nc.sync.dma_start(out=xq_sb[dt][1][:], in_=xq_d[dt * 128:(dt + 1) * 128, CH:2 * CH])
        issue_eb(1, 0)

        # pair-1 kT + qT(c0) then V for pair 0 (A(0,0) needs it)
        for c0 in range(0, tk, CH):
            proj_qk(wk_sb, lambda dt, c: (xkv_sb[dt], c), kT_sb, 1, c0)
        proj_qk(wq_sb, lambda dt, c: (xq_sb[dt][0], 0), qT_sb, 1, 0)
        for t in range(ntk):
            vproj(t, 0)

        for hh in range(2):
            attnv_norm(0, 0, hh)
            drop_p(0, 0, hh)

        scores_group(1, 0)
        issue_eb(0, 1)
        # V pair 1 + qT second halves fill the exp window
        for t in range(ntk):
            vproj(t, 1)
        proj_qk(wq_sb, lambda dt, c: (xq_sb[dt][1], 0), qT_sb, 0, CH)
        proj_qk(wq_sb, lambda dt, c: (xq_sb[dt][1], 0), qT_sb, 1, CH)
        for hh in range(2):
            attnv_norm(1, 0, hh)
            drop_p(1, 0, hh)

        scores_group(0, 1)
        issue_eb(1, 1)
        outproj(0, CH // 128, copy_eng=nc.scalar)
        for hh in range(2):
            attnv_norm(0, 1, hh)
            drop_p(0, 1, hh)

        # last group per-head: attnV/norm of hh0 overlaps hh1's exp stream
        scores_group(1, 1, hhs=(0,))
        scores_group(1, 1, hhs=(1,))
        attnv_norm(1, 1, 0)
        drop_p(1, 1, 0)
        attnv_norm(1, 1, 1)
        drop_p(1, 1, 1)
        outproj(CH // 128, tq // 128)

    nc.compile()
    return nc


_NC = None
LAST_RESULTS = None


def _get_nc():
    global _NC
    if _NC is None:
        _NC = build_nc()
    return _NC


def _shard_inputs(query, key_value, mask, rel_pos_bias, Wq, Wkv, Wo):
    """Build the 8 per-core input maps (host-side transposes + exp-bias)."""
    in_maps = []
    ndt = D_MODEL // 128
    pairs = HPC // 2
    nch = TQ // CH
    ntk = TK // 128
    w_f16 = {
        "Wq": Wq.astype(NP_F16),
        "Wo": Wo.astype(NP_F16),
        "Wkv": Wkv.astype(NP_F16),
    }

    def wmat(w):  # [D, hd] -> [128, ndt, hd] partition-major
        return np.ascontiguousarray(
            w.reshape(ndt, 128, HPC * D_HEAD).transpose(1, 0, 2))

    for c in range(N_CORES):
        b = c // (N_CORES // B)
        g = c % (N_CORES // B)
        cs = slice(g * HPC * D_HEAD, (g + 1) * HPC * D_HEAD)
        hs = slice(g * HPC, (g + 1) * HPC)
        # expb = exp(bias)^T * mask^T  ->  [pair, chunk, t, 128, hh*CH]
        eb = np.exp(rel_pos_bias[hs].astype(np.float32)).transpose(0, 2, 1)
        eb = eb * mask[b, 0].T[None].astype(np.float32)
        eb = eb.astype(NP_F16)                      # [4, tk, tq]
        eb = eb.reshape(pairs, 2, ntk, 128, nch, CH)
        eb = np.ascontiguousarray(eb.transpose(0, 4, 2, 3, 1, 5))
        wo = w_f16["Wo"][cs, :]                     # [hd, D]
        wo = np.ascontiguousarray(
            wo.reshape(pairs, 128, D_MODEL).transpose(1, 0, 2))
        in_maps.append({
            "xqT": np.ascontiguousarray(query[b].T).astype(NP_F16),
            "xkvT": np.ascontiguousarray(key_value[b].T).astype(NP_F16),
            "wq": wmat(w_f16["Wq"][:, cs]),
            "wk": wmat(w_f16["Wkv"][:, cs]),
            "wv": wmat(w_f16["Wkv"][:, D_MODEL + cs.start:D_MODEL + cs.stop]),
            "wo": wo,
            "expb": eb.reshape(pairs, nch, ntk, 128, 2 * CH),
        })
    return in_maps


def kernel(query, key_value, mask, rel_pos_bias, Wq, Wkv, Wo):
    global LAST_RESULTS
    query, key_value, mask, rel_pos_bias, Wq, Wkv, Wo = (
        np.asarray(a) for a in (query, key_value, mask, rel_pos_bias, Wq, Wkv, Wo)
    )
    nc = _get_nc()
    in_maps = _shard_inputs(query, key_value, mask, rel_pos_bias, Wq, Wkv, Wo)
    res = run_bass_kernel_spmd(nc, in_maps, core_ids=list(range(N_CORES)))
    LAST_RESULTS = res
    gpc = N_CORES // B  # cores per batch group
    out = np.stack([
        sum(res.results[b * gpc + i]["out"].astype(np.float32) for i in range(gpc))
        for b in range(B)
    ])
    return out


# revision 23
# speedup vs baseline: 1.5538x; 1.5538x over previous
"""MultiHeadCrossAttention Trainium2 Bass kernel.

Sharding (8 cores): data-parallel over batch (2) x tensor-parallel over
head groups (4 groups of 4 heads).  Core c handles batch c//4, heads
4*(c%4) .. 4*(c%4)+3.  Each core computes a partial [Tq, D] output
(its heads' contribution through its Wo row-slice); the host sums the 4
partials per batch.

Device math per core (all matmuls fp16 x fp16 -> fp32 PSUM):
  qT = Wq_s.T @ Xq.T          [256, Tq]   (head-dim on partitions)
  kT = Wk_s.T @ Xkv.T         [256, Tk]
  V  = Xkv @ Wv_s             [Tk, 256]   (+ ones column per head)
  St = kT_h.T @ qT_h          [Tk, Tq] scores^T, K=64, head pairs packed
                              into PE row-groups 0-63 / 64-127
  E  = exp(St/8)              (ScalarE, scale folded into activation)
  P  = E * expb               expb = exp(bias^T) * mask^T  (host-built;
                              multiplicative bias: exp(s+b) = exp(s)exp(b))
  [out^T; sums] = [V_h|1].T @ P   [65, Tq]  ones-column gives softmax sums
  out_norm^T = out^T * (1/sums)   (one batched reciprocal per tq chunk,
                                   then gpsimd partition_broadcast)
  partial = stack(out_norm^T).T @ Wo_s      [Tq, D]  (fp16 out, host sums)

Softmax max-subtraction is skipped: logits ~ N(0, ~1.1), max |logit| < ~7
over 16M samples, exp stays in fp16/fp32 range comfortably.
"""

import os
from contextlib import ExitStack

import numpy as np

import concourse.bass as bass
import concourse.mybir as mybir
import concourse.tile as tile
from concourse import bacc
from concourse.bass_utils import run_bass_kernel_spmd

# Problem dims (hardcoded per contract).
D_MODEL = 1024
NUM_HEADS = 16
D_HEAD = 64
B = 2
TQ = 2048
TK = 2048
N_CORES = 8
HPC = 4  # heads per core
SCALE = 1.0 / 8.0  # 1/sqrt(D_HEAD)

F16 = mybir.dt.float16
F32 = mybir.dt.float32
NP_F16 = np.float16

NQ = 512  # matmul moving free-dim chunk (PSUM bank = 512 fp32)


def build_nc(d_model=D_MODEL, tq=TQ, tk=TK, hpc=HPC, d_head=D_HEAD, scale=SCALE):
    """Build the single-core Bass program (SPMD: same NEFF on all cores)."""
    assert d_model % 128 == 0 and tq % NQ == 0 and tk % 128 == 0
    assert hpc % 2 == 0
    ndt = d_model // 128          # contraction tiles for projections
    pairs = hpc // 2              # head pairs (128 head-dims per pair)
    hd = hpc * d_head             # per-core head dims (= 256)
    ntq = tq // NQ                # Tq chunks of 512
    ntk = tk // 128               # Tk tiles of 128
    vw = d_head + 1               # V columns per head incl. ones column
    CH = min(tq, 1024)            # scores psum tile width (2 PSUM banks)
    nqc = CH // NQ                # 512-chunks per scores tile
    n_tqh = tq // CH              # tq macro-chunks per head

    nc = bacc.Bacc("TRN2", target_bir_lowering=False, debug=False)

    xq_d = nc.dram_tensor("xqT", [d_model, tq], F16, kind="ExternalInput")
    xkv_d = nc.dram_tensor("xkvT", [d_model, tk], F16, kind="ExternalInput")
    # weights shipped partition-major (4KB contiguous per partition line)
    wq_d = nc.dram_tensor("wq", [128, ndt, hd], F16, kind="ExternalInput")
    wk_d = nc.dram_tensor("wk", [128, ndt, hd], F16, kind="ExternalInput")
    wv_d = nc.dram_tensor("wv", [128, ndt, hd], F16, kind="ExternalInput")
    wo_d = nc.dram_tensor("wo", [128, pairs, d_model], F16, kind="ExternalInput")
    # [pair, tq-chunk, t, partition(=tk%128), hh*CH] — each (pair,chunk,t)
    # block is one contiguous [128, 2*CH] transfer with 4KB lines
    eb_d = nc.dram_tensor("expb", [pairs, tq // CH, ntk, 128, 2 * CH], F16,
                          kind="ExternalInput")
    out_d = nc.dram_tensor("out", [tq, d_model], F16, kind="ExternalOutput")

    with ExitStack() as ctx:
        tc = ctx.enter_context(tile.TileContext(nc))
        # ---- persistent pools
        wpool = ctx.enter_context(tc.tile_pool(name="wpool", bufs=1))
        qkpool = ctx.enter_context(tc.tile_pool(name="qkpool", bufs=1))
        opool = ctx.enter_context(tc.tile_pool(name="opool", bufs=3))
        npool = ctx.enter_context(tc.tile_pool(name="npool", bufs=4))
        upool = ctx.enter_context(tc.tile_pool(name="upool", bufs=hpc * ntq))
        psS = ctx.enter_context(tc.tile_pool(name="psS", bufs=3, space="PSUM"))
        psO = ctx.enter_context(tc.tile_pool(name="psO", bufs=2, space="PSUM"))

        wq_sb = wpool.tile([128, ndt, hd], F16, tag="wq")
        wk_sb = wpool.tile([128, ndt, hd], F16, tag="wk")
        wv_sb = wpool.tile([128, ndt, hd], F16, tag="wv")
        wo_sb = wpool.tile([128, pairs, d_model], F16, tag="wo")
        nc.sync.dma_start(out=wk_sb[:], in_=wk_d.ap())
        nc.sync.dma_start(out=wv_sb[:], in_=wv_d.ap())

        qT_sb = qkpool.tile([128, pairs, tq], F16, tag="qT")
        kT_sb = qkpool.tile([128, pairs, tk], F16, tag="kT")
        v_sb = qkpool.tile([128, ntk, hpc * vw], F16, tag="v")
        stack_sb = qkpool.tile([128, pairs, tq], F16, tag="stack")

        # ones columns of v_sb (projection copies overwrite the V columns)
        nc.gpsimd.memset(v_sb[:], 1.0)

        # ---- phase A: projections (X^T resident only here)
        with tc.tile_pool(name="xpool", bufs=1) as xpool:
            # one tile per d-slice so each projection matmul depends only on
            # its own 0.5 MB DMA (kv first: kT, V and scores need it)
            xkv_sb = [xpool.tile([128, tk], F16, tag=f"xkv{dt}", name="xkv_sb") for dt in range(ndt)]
            xq_sb = [xpool.tile([128, tq], F16, tag=f"xq{dt}", name="xq_sb") for dt in range(ndt)]
            for dt in range(ndt):
                nc.sync.dma_start(out=xkv_sb[dt][:], in_=xkv_d[dt * 128 : (dt + 1) * 128, :])
            nc.sync.dma_start(out=wq_sb[:], in_=wq_d.ap())
            for dt in range(ndt):
                nc.sync.dma_start(out=xq_sb[dt][:], in_=xq_d[dt * 128 : (dt + 1) * 128, :])
            nc.sync.dma_start(out=wo_sb[:], in_=wo_d.ap())

            # qT / kT: [j-pair 128, tq]  = sum_d W[:, j].T @ X^T
            for wsb, xsb, dst, tlen in ((wk_sb, xkv_sb, kT_sb, tk), (wq_sb, xq_sb, qT_sb, tq)):
                for j in range(pairs):
                    for c0 in range(0, tlen, CH):
                        cn = min(CH, tlen - c0)
                        ps = psS.tile([128, cn], F32, tag="ps", name="ps")
                        for dt in range(ndt):
                            for q0 in range(0, cn, NQ):
                                qn = min(NQ, cn - q0)
                                nc.tensor.matmul(
                                    ps[:, q0 : q0 + qn],
                                    wsb[:, dt, j * 128 : (j + 1) * 128],
                                    xsb[dt][:, c0 + q0 : c0 + q0 + qn],
                                    start=(dt == 0),
                                    stop=(dt == ndt - 1),
                                )
                        nc.vector.tensor_copy(dst[:, j, c0 : c0 + cn], ps[:])

            # V: [tk 128, hd] = X_kv @ Wv ; scatter per head next to ones cols
            for t in range(ntk):
                psv = psO.tile([128, hd], F32, tag="po", name="psv")
                for dt in range(ndt):
                    nc.tensor.matmul(
                        psv[:],
                        xkv_sb[dt][:, t * 128 : (t + 1) * 128],
                        wv_sb[:, dt, :],
                        start=(dt == 0),
                        stop=(dt == ndt - 1),
                    )
                nc.vector.tensor_copy(
                    v_sb[:, t, :].rearrange("p (h w) -> p h w", w=vw)[:, :, 0:d_head],
                    psv[:].rearrange("p (h w) -> p h w", w=d_head),
                )

        # ---- phase B + C: attention pipelined with normalize/out-projection.
        # tqh outer so each tq macro-chunk finishes all heads, then its
        # normalize + out-projection overlap the next chunk's attention.
        with (
            tc.tile_pool(name="ppool", bufs=2 * ntk + 12) as ppool,
            tc.tile_pool(name="ebpool", bufs=4) as ebpool,
        ):
            for tqh in range(n_tqh):
                c0 = tqh * CH
                for pair in range(pairs):
                    # scores^T + exp + expb-mul for both heads of the pair
                    p_ts = []
                    for t in range(ntk):
                        tr = slice(t * 128, (t + 1) * 128)
                        eb_t = ebpool.tile([128, 2, CH], F16, tag="eb", name="eb")
                        nc.sync.dma_start(out=eb_t[:], in_=eb_d[pair, tqh, t])
                        psAB = []
                        for hh in range(2):
                            psAB.append(psS.tile([128, CH], F32, tag="ps", name="ps"))
                        for q0 in range(0, CH, NQ):
                            for hh in range(2):
                                r0 = hh * 64
                                nc.tensor.matmul(
                                    psAB[hh][:, q0 : q0 + NQ],
                                    kT_sb[r0 : r0 + 64, pair, tr],
                                    qT_sb[r0 : r0 + 64, pair, c0 + q0 : c0 + q0 + NQ],
                                    start=True,
                                    stop=True,
                                )
                        pp = []
                        for hh in range(2):
                            p_t = ppool.tile([128, CH], F16, tag="p", name="p_t")
                            nc.scalar.activation(
                                out=p_t[:], in_=psAB[hh][:],
                                func=mybir.ActivationFunctionType.Exp, scale=scale,
                            )
                            nc.vector.tensor_mul(p_t[:], p_t[:], eb_t[:, hh, :])
                            pp.append(p_t)
                        p_ts.append(pp)

                    # attn @ [V|1] -> [65, NQ] per (head, 512-chunk)
                    for hh in range(2):
                        h = 2 * pair + hh
                        po = [psO.tile([vw, NQ], F32, tag="po", name="po") for _ in range(nqc)]
                        for t in range(ntk):
                            for qi in range(nqc):
                                nc.tensor.matmul(
                                    po[qi][:],
                                    v_sb[:, t, h * vw : (h + 1) * vw],
                                    p_ts[t][hh][:, qi * NQ : (qi + 1) * NQ],
                                    start=(t == 0),
                                    stop=(t == ntk - 1),
                                )
                        for qi in range(nqc):
                            qg = tqh * nqc + qi  # global 512-chunk index
                            u_t = upool.tile([64, NQ], F16, tag="u", name="u_t")
                            nc.vector.tensor_copy(u_t[:], po[qi][0:64, :])
                            sm_t = npool.tile([1, NQ], F32, tag="sm", name="sm_t")
                            nc.vector.tensor_copy(sm_t[:], po[qi][64:65, :])
                            # normalize: fast approx reciprocal (no DMA
                            # roundtrips / slow iterative reciprocal)
                            smr = npool.tile([1, NQ], F32, tag="smr", name="smr")
                            nc.vector.reciprocal_approx_fast(out=smr[:], in_=sm_t[:])
                            smr16 = npool.tile([1, NQ], F16, tag="smr16", name="smr16")
                            nc.vector.tensor_copy(smr16[:], smr[:])
                            rb_t = npool.tile([64, NQ], F16, tag="rb", name="rb_t")
                            nc.gpsimd.partition_broadcast(rb_t[:], smr16[:])
                            nc.vector.tensor_mul(
                                stack_sb[hh * 64 : hh * 64 + 64, pair,
                                         qg * NQ : (qg + 1) * NQ],
                                u_t[:],
                                rb_t[:],
                            )

                # out-projection for this tq chunk
                for ti in range(CH // 128):
                    t = tqh * (CH // 128) + ti
                    last = tqh == n_tqh - 1
                    osb = opool.tile([128, d_model], F16, tag="osb", name="osb")
                    for mc0 in range(0, d_model, CH if last else NQ):
                        mcn = min(CH if last else NQ, d_model - mc0)
                        if last:
                            pf = psS.tile([128, mcn], F32, tag="ps", name="pf")
                        else:
                            pf = psO.tile([128, mcn], F32, tag="po", name="pf")
                        for pair in range(pairs):
                            for m0 in range(0, mcn, NQ):
                                mn = min(NQ, mcn - m0)
                                nc.tensor.matmul(
                                    pf[:, m0 : m0 + mn],
                                    stack_sb[:, pair, t * 128 : (t + 1) * 128],
                                    wo_sb[:, pair, mc0 + m0 : mc0 + m0 + mn],
                                    start=(pair == 0),
                                    stop=(pair == pairs - 1),
                                )
                        eng = nc.vector.tensor_copy if ti % 2 == 0 else nc.scalar.copy
                        eng(osb[:, mc0 : mc0 + mcn], pf[:])
                    nc.sync.dma_start(out=out_d[t * 128 : (t + 1) * 128, :], in_=osb[:])

    nc.compile()
    return nc


_NC = None
LAST_RESULTS = None


def _get_nc():
    global _NC
    if _NC is None:
        _NC = build_nc()
    return _NC


def _shard_inputs(query, key_value, mask, rel_pos_bias, Wq, Wkv, Wo):
    """Build the 8 per-core input maps (host-side transposes + exp-bias)."""
    in_maps = []
    ndt = D_MODEL // 128
    pairs = HPC // 2
    CH = min(TQ, 1024)
    nch = TQ // CH
    ntk = TK // 128
    w_f16 = {
        "Wq": Wq.astype(NP_F16),
        "Wo": Wo.astype(NP_F16),
        "Wkv": Wkv.astype(NP_F16),
    }

    def wmat(w):  # [D, hd] -> [128, ndt, hd] partition-major
        return np.ascontiguousarray(
            w.reshape(ndt, 128, HPC * D_HEAD).transpose(1, 0, 2))

    for c in range(N_CORES):
        b = c // (N_CORES // B)
        g = c % (N_CORES // B)
        cs = slice(g * HPC * D_HEAD, (g + 1) * HPC * D_HEAD)
        hs = slice(g * HPC, (g + 1) * HPC)
        # expb = exp(bias)^T * mask^T -> [pair, chunk, t, 128, hh*CH]
        eb = np.exp(rel_pos_bias[hs].astype(np.float32)).transpose(0, 2, 1)
        eb = eb * mask[b, 0].T[None].astype(np.float32)
        eb = eb.astype(NP_F16)                      # [4, tk, tq]
        eb = eb.reshape(pairs, 2, ntk, 128, nch, CH)
        eb = np.ascontiguousarray(eb.transpose(0, 4, 2, 3, 1, 5))
        wo = w_f16["Wo"][cs, :]                     # [hd, D]
        wo = np.ascontiguousarray(
            wo.reshape(pairs, 128, D_MODEL).transpose(1, 0, 2))
        in_maps.append({
            "xqT": np.ascontiguousarray(query[b].T).astype(NP_F16),
            "xkvT": np.ascontiguousarray(key_value[b].T).astype(NP_F16),
            "wq": wmat(w_f16["Wq"][:, cs]),
            "wk": wmat(w_f16["Wkv"][:, cs]),
            "wv": wmat(w_f16["Wkv"][:, D_MODEL + cs.start : D_MODEL + cs.stop]),
            "wo": wo,
            "expb": eb.reshape(pairs, nch, ntk, 128, 2 * CH),
        })
    return in_maps


def kernel(query, key_value, mask, rel_pos_bias, Wq, Wkv, Wo):
    global LAST_RESULTS
    query, key_value, mask, rel_pos_bias, Wq, Wkv, Wo = (
        np.asarray(a) for a in (query, key_value, mask, rel_pos_bias, Wq, Wkv, Wo)
    )
    nc = _get_nc()
    in_maps = _shard_inputs(query, key_value, mask, rel_pos_bias, Wq, Wkv, Wo)
    res = run_bass_kernel_spmd(nc, in_maps, core_ids=list(range(N_CORES)))
    LAST_RESULTS = res
    gpc = N_CORES // B  # cores per batch group
    out = np.stack([
        sum(res.results[b * gpc + i]["out"].astype(np.float32) for i in range(gpc))
        for b in range(B)
    ])
    return out

